# revision 23
# baseline (speedup 1.0000x reference)
"""BiAttention (mode==1) Trainium2 Bass kernel.

Reference computation (per batch b, for (W,bias) in [(W2,b2),(W3,b3)]):
    proj   = input2[b] @ W.T + bias          # [S, D]
    scores = input1[b] @ proj.T              # [T, S]
    w      = softmax(scores, axis=-1)
    out    = w @ input2[b]                   # [T, D]
with B=16, T=2048, S=1024, D=300.

Key restructurings (validated vs reference in fp64/fp32):
  * The bias contributes sum_e bias[e]*input1[b,t,e] to scores — constant in s,
    so it cancels in softmax and is dropped entirely.
  * Everything is computed in the transposed "scoresT" orientation [s, t] so
    that every matmul contracts over the partition dim with NO on-chip
    transposes:
        projT  [e, s] = Wt.T @ input2T      (lhsT = W.T padded, rhs = input2T)
        scoresT[s, t] = projT.T @ input1T   (lhsT = projT slices, rhs = input1T)
        E = exp(scoresT)                    (no max-subtraction: |scores| < ~60)
        out[t, :304]  = E.T @ [input2 | 1]  (lhsT = E slices, rhs = input2
                                             augmented with a ones column, so
                                             column 300 accumulates sum_s E =
                                             the softmax denominator for free)
        out[t, d] = out[t, d] / out[t, 300]
  * input1T / input2T / W.T are prepared on the host (numpy) and passed as the
    kernel's DRAM inputs; D=300 is zero-padded to 384 = 3*128 K-chunks.
  * Data-parallel over batch: 8 cores x 2 batches each, params replicated.
"""

import os

import numpy as np

B, T, S, D = 16, 2048, 1024, 300
DP = 384          # D padded to 3 K-chunks of 128
NA = 304          # input2 free dim: 300 data + ones col at 300 + pad
                  # (301 fails walrus "ISA check" on the f32r matmul)
NB = 2            # batches per core
NCORES = 8

_CACHE = {}


def _split_multi_waits(nc, maxw=1):
    """This walrus/CoreV3 build accepts at most one semaphore sync-wait per
    instruction ("Too many sync wait commands").  Tile attaches several to
    matmuls/DMAs/the tail Drain.  Post-scheduling, splice NOP carrier
    instructions (one wait each) in front of any instruction with more."""
    import concourse.mybir as mybir

    ctr = 0
    for fn in nc.m.functions:
        for blk in fn.blocks:
            insts = blk.instructions
            i = 0
            while i < len(insts):
                inst = insts[i]
                si = getattr(inst, "sync_info", None)
                waits = list(si.on_wait) if si is not None and si.on_wait else []
                if len(waits) > maxw:
                    si.on_wait = waits[len(waits) - maxw :]
                    carriers = []
                    for w in waits[: len(waits) - maxw]:
                        ctr += 1
                        carriers.append(
                            mybir.InstNoOp(
                                name=f"waitsplit-{ctr}",
                                engine=inst.engine,
                                ins=[],
                                outs=[],
                                sync_info=mybir.SyncInfo(on_wait=[w], on_update=[]),
                                bass_nofuse=True,
                            )
                        )
                    insts[i:i] = carriers
                    i += len(carriers)
                i += 1


def _install_profile_hook():
    """Synthesize the missing ``antenv.axon_hooks`` glue so run_bass_kernel_spmd
    trace=True can drive NTFF profiling through the injected libaxon_pjrt.so,
    and stub out the artifact upload (no bucket access here)."""
    import sys
    import types

    if "antenv.axon_hooks" not in sys.modules:
        mod = types.ModuleType("antenv.axon_hooks")
        holder = {}
        mod.set_axon_ntff_profile_hook = lambda h: holder.__setitem__("h", h)
        mod.get_axon_ntff_profile_hook = lambda: holder.get("h")
        sys.modules["antenv.axon_hooks"] = mod
        try:
            from trn_agent_boot.trn_boot import _ntff_profile_via_ctypes

            mod.set_axon_ntff_profile_hook(
                _ntff_profile_via_ctypes("/opt/axon/libaxon_pjrt.so")
            )
        except Exception:
            pass

    import concourse.bass_utils as bu

    if not getattr(bu, "_upload_stubbed", False):
        bu.upload_artifacts = lambda tmpdir: f"local:{tmpdir}"
        bu._upload_stubbed = True


def _build_nc():
    import concourse.bass as bass
    import concourse.mybir as mybir
    from concourse.tile import TileContext

    f32 = mybir.dt.float32
    # float32r streams fp32 data through the PE at 1 cycle/row (vs 4 for
    # plain fp32's two half-rate passes) when the moving dim is >=256; HW
    # probe: absmax rel err 1.6e-4 on K=128 dots (vs 2.2e-3 for bf16).
    f32r = mybir.dt.float32r
    bf16 = mybir.dt.bfloat16
    Exp = mybir.ActivationFunctionType.Exp

    NT = T // 512   # 4 t-blocks of 512
    NS = S // 128   # 8 s-chunks of 128

    nc = bass.Bass("TRN2", target_bir_lowering=False, debug=False)
    in1t = nc.declare_dram_parameter("in1t", [NB, DP, T], f32r, isOutput=False)
    in2t = nc.declare_dram_parameter("in2t", [NB, 2, 128, 3, 512], f32r, isOutput=False)
    in2n = nc.declare_dram_parameter("in2n", [NB, 128, NS, NA], bf16, isOutput=False)
    wts = nc.declare_dram_parameter("wts", [2, 128, 3, DP], f32r, isOutput=False)
    out_h = [
        nc.declare_dram_parameter("out_a", [NB, T // 512, 128, 4, D], f32, isOutput=True),
        nc.declare_dram_parameter("out_b", [NB, T // 512, 128, 4, D], f32, isOutput=True),
    ]

    with TileContext(nc) as tc:
        with (
            tc.tile_pool(name="wpool", bufs=1) as wpool,
            tc.tile_pool(name="a1p", bufs=2) as a1p,
            tc.tile_pool(name="a2p", bufs=2) as a2p,
            tc.tile_pool(name="a2np", bufs=2) as a2np,
            tc.tile_pool(name="projp", bufs=2) as projp,
            tc.tile_pool(name="ep", bufs=2) as ep,
            tc.tile_pool(name="outp", bufs=3) as outp,
            tc.tile_pool(name="recp", bufs=4) as recp,
            tc.tile_pool(name="ps_pj", bufs=2, space="PSUM") as ps_pj,
            tc.tile_pool(name="ps_sc", bufs=3, space="PSUM") as ps_sc,
            tc.tile_pool(name="ps_o", bufs=2, space="PSUM") as ps_o,
        ):
            # Weights: [p, attn, kd, e] — one DMA, resident all kernel.
            wt = wpool.tile([128, 2, 3, DP], f32r)
            nc.sync.dma_start(out=wt[:, 0, :, :], in_=wts[0])

            for lb in range(NB):
                # Load order matters for the pipeline head: mm1 needs only
                # a2 (+wt); mm2 then consumes a1 chunk-by-chunk; a2n is not
                # needed until the first mm4 (~15us of PE work later).
                a2 = a2p.tile([128, 2, 3, 512], f32r)
                for h in range(2):
                    nc.sync.dma_start(out=a2[:, h], in_=in2t[lb, h])
                if lb == 0:
                    nc.sync.dma_start(out=wt[:, 1, :, :], in_=wts[1])
                a1 = a1p.tile([128, 3, T], f32r)
                for c in range(3):
                    nc.sync.dma_start(
                        out=a1[:, c, :],
                        in_=in1t[lb, c * 128 : (c + 1) * 128, :],
                    )
                a2n = a2np.tile([128, NS, NA], bf16)
                nc.sync.dma_start(out=a2n, in_=in2n[lb])

                for a in range(2):
                    # mm1: projT [e, s] in 3 e-chunks of 128.
                    pt = projp.tile([128, 3, S], f32r)
                    for h in range(S // 512):
                        for ke in range(3):
                            pj = ps_pj.tile([128, 512], f32)
                            for kd in range(3):
                                nc.tensor.matmul(
                                    pj,
                                    wt[:, a, kd, ke * 128 : (ke + 1) * 128],
                                    a2[:, h, kd, :],
                                    start=(kd == 0),
                                    stop=(kd == 2),
                                )
                            nc.vector.tensor_copy(
                                pt[:, ke, h * 512 : (h + 1) * 512], pj
                            )

                    for tb in range(NT):
                        # mm2 + exp: E[s, t] for this 512-wide t-block.
                        E = ep.tile([128, NS, 512], bf16)
                        for st in range(NS):
                            sc = ps_sc.tile([128, 512], f32)
                            for ke in range(3):
                                nc.tensor.matmul(
                                    sc,
                                    pt[:, ke, st * 128 : (st + 1) * 128],
                                    a1[:, ke, tb * 512 : (tb + 1) * 512],
                                    start=(ke == 0),
                                    stop=(ke == 2),
                                )
                            nc.scalar.activation(out=E[:, st, :], in_=sc, func=Exp)

                        # mm4 + normalize: out[t, :300] and denominator col 300.
                        ostg = outp.tile([128, 4, D], f32)
                        for ts in range(4):
                            o = ps_o.tile([128, NA], f32)
                            for st in range(NS):
                                nc.tensor.matmul(
                                    o,
                                    E[:, st, ts * 128 : (ts + 1) * 128],
                                    a2n[:, st, :],
                                    start=(st == 0),
                                    stop=(st == NS - 1),
                                )
                            rec = recp.tile([128, 1], f32)
                            nc.vector.reciprocal(rec, o[:, 300:301])
                            nc.vector.tensor_scalar_mul(
                                ostg[:, ts, :], o[:, 0:D], rec
                            )
                        for ts in range(4):
                            nc.sync.dma_start(
                                out=out_h[a][lb, tb, :, ts], in_=ostg[:, ts]
                            )
    _split_multi_waits(nc)
    return nc


def kernel(input1, input2, W2, b2, W3, b3, mode=None, **_ignored):
    from concourse.bass_utils import run_bass_kernel_spmd

    input1 = np.asarray(input1, dtype=np.float32)
    input2 = np.asarray(input2, dtype=np.float32)
    W2 = np.asarray(W2, dtype=np.float32)
    W3 = np.asarray(W3, dtype=np.float32)
    # bias b2/b3 add a per-(b,t) constant to the softmax logits — no effect.

    if "nc" not in _CACHE:
        _CACHE["nc"] = _build_nc()
    nc = _CACHE["nc"]

    in1t = np.zeros((B, DP, T), np.float32)
    in1t[:, :D, :] = input1.transpose(0, 2, 1)
    in2t = np.zeros((B, DP, S), np.float32)
    in2t[:, :D, :] = input2.transpose(0, 2, 1)
    # [B, d(c*128+p), s(h*512+j)] -> [B, h, p, c, j]
    in2t = np.ascontiguousarray(
        in2t.reshape(B, 3, 128, 2, 512).transpose(0, 3, 2, 1, 4)
    )
    import ml_dtypes

    in2n = np.zeros((B, S, NA), np.float32)
    in2n[:, :, :D] = input2
    in2n[:, :, 300] = 1.0
    in2n = np.ascontiguousarray(
        in2n.reshape(B, S // 128, 128, NA).transpose(0, 2, 1, 3)
    ).astype(ml_dtypes.bfloat16)
    wts = np.zeros((2, DP, DP), np.float32)
    wts[0, :D, :D] = W2.T
    wts[1, :D, :D] = W3.T
    wts = np.ascontiguousarray(wts.reshape(2, 3, 128, DP).transpose(0, 2, 1, 3))

    in_maps = [
        {
            "in1t": np.ascontiguousarray(in1t[c * NB : (c + 1) * NB]),
            "in2t": np.ascontiguousarray(in2t[c * NB : (c + 1) * NB]),
            "in2n": np.ascontiguousarray(in2n[c * NB : (c + 1) * NB]),
            "wts": wts,
        }
        for c in range(NCORES)
    ]

    trace = bool(int(os.environ.get("KERNEL_PROFILE", "0")))
    if trace:
        _install_profile_hook()
    res = run_bass_kernel_spmd(nc, in_maps, list(range(NCORES)), trace=trace)
    _CACHE["last_exec_time_ns"] = res.exec_time_ns
    _CACHE["last_results"] = res

    def unswizzle(name):
        arr = np.concatenate([res.results[c][name] for c in range(NCORES)], axis=0)
        # [B, T//512, 128(p), 4(ts), D] -> [B, T, D] with t = tb*512 + ts*128 + p
        return np.ascontiguousarray(
            arr.transpose(0, 1, 3, 2, 4).reshape(B, T, D)
        )

    return unswizzle("out_a"), unswizzle("out_b")


# revision 24
# speedup vs baseline: 1.0308x; 1.0308x over previous
"""BiAttention (mode==1) Trainium2 Bass kernel.

Reference computation (per batch b, for (W,bias) in [(W2,b2),(W3,b3)]):
    proj   = input2[b] @ W.T + bias          # [S, D]
    scores = input1[b] @ proj.T              # [T, S]
    w      = softmax(scores, axis=-1)
    out    = w @ input2[b]                   # [T, D]
with B=16, T=2048, S=1024, D=300.

Key restructurings (validated vs reference in fp64/fp32):
  * The bias contributes sum_e bias[e]*input1[b,t,e] to scores — constant in s,
    so it cancels in softmax and is dropped entirely.
  * Everything is computed in the transposed "scoresT" orientation [s, t] so
    that every matmul contracts over the partition dim with NO on-chip
    transposes:
        projT  [e, s] = Wt.T @ input2T      (lhsT = W.T padded, rhs = input2T)
        scoresT[s, t] = projT.T @ input1T   (lhsT = projT slices, rhs = input1T)
        E = exp(scoresT)                    (no max-subtraction: |scores| < ~60)
        out[t, :304]  = E.T @ [input2 | 1]  (lhsT = E slices, rhs = input2
                                             augmented with a ones column, so
                                             column 300 accumulates sum_s E =
                                             the softmax denominator for free)
        out[t, d] = out[t, d] / out[t, 300]
  * input1T / input2T / W.T are prepared on the host (numpy) and passed as the
    kernel's DRAM inputs; D=300 is zero-padded to 384 = 3*128 K-chunks.
  * Data-parallel over batch: 8 cores x 2 batches each, params replicated.
"""

import os

import numpy as np

B, T, S, D = 16, 2048, 1024, 300
DP = 384          # D padded to 3 K-chunks of 128
NA = 304          # input2 free dim: 300 data + ones col at 300 + pad
                  # (301 fails walrus "ISA check" on the f32r matmul)
NB = 2            # batches per core
NCORES = 8

_CACHE = {}


def _split_multi_waits(nc, maxw=1):
    """This walrus/CoreV3 build accepts at most one semaphore sync-wait per
    instruction ("Too many sync wait commands").  Tile attaches several to
    matmuls/DMAs/the tail Drain.  Post-scheduling, splice NOP carrier
    instructions (one wait each) in front of any instruction with more."""
    import concourse.mybir as mybir

    ctr = 0
    for fn in nc.m.functions:
        for blk in fn.blocks:
            insts = blk.instructions
            i = 0
            while i < len(insts):
                inst = insts[i]
                si = getattr(inst, "sync_info", None)
                waits = list(si.on_wait) if si is not None and si.on_wait else []
                if len(waits) > maxw:
                    si.on_wait = waits[len(waits) - maxw :]
                    carriers = []
                    for w in waits[: len(waits) - maxw]:
                        ctr += 1
                        carriers.append(
                            mybir.InstNoOp(
                                name=f"waitsplit-{ctr}",
                                engine=inst.engine,
                                ins=[],
                                outs=[],
                                sync_info=mybir.SyncInfo(on_wait=[w], on_update=[]),
                                bass_nofuse=True,
                            )
                        )
                    insts[i:i] = carriers
                    i += len(carriers)
                i += 1


def _install_profile_hook():
    """Synthesize the missing ``antenv.axon_hooks`` glue so run_bass_kernel_spmd
    trace=True can drive NTFF profiling through the injected libaxon_pjrt.so,
    and stub out the artifact upload (no bucket access here)."""
    import sys
    import types

    if "antenv.axon_hooks" not in sys.modules:
        mod = types.ModuleType("antenv.axon_hooks")
        holder = {}
        mod.set_axon_ntff_profile_hook = lambda h: holder.__setitem__("h", h)
        mod.get_axon_ntff_profile_hook = lambda: holder.get("h")
        sys.modules["antenv.axon_hooks"] = mod
        try:
            from trn_agent_boot.trn_boot import _ntff_profile_via_ctypes

            mod.set_axon_ntff_profile_hook(
                _ntff_profile_via_ctypes("/opt/axon/libaxon_pjrt.so")
            )
        except Exception:
            pass

    import concourse.bass_utils as bu

    if not getattr(bu, "_upload_stubbed", False):
        bu.upload_artifacts = lambda tmpdir: f"local:{tmpdir}"
        bu._upload_stubbed = True


def _build_nc():
    import concourse.bass as bass
    import concourse.mybir as mybir
    from concourse.tile import TileContext

    f32 = mybir.dt.float32
    # float32r streams fp32 data through the PE at 1 cycle/row (vs 4 for
    # plain fp32's two half-rate passes) when the moving dim is >=256; HW
    # probe: absmax rel err 1.6e-4 on K=128 dots (vs 2.2e-3 for bf16).
    f32r = mybir.dt.float32r
    bf16 = mybir.dt.bfloat16
    Exp = mybir.ActivationFunctionType.Exp

    NT = T // 512   # 4 t-blocks of 512
    NS = S // 128   # 8 s-chunks of 128

    nc = bass.Bass("TRN2", target_bir_lowering=False, debug=False)
    in1t = nc.declare_dram_parameter("in1t", [NB, DP, T], f32r, isOutput=False)
    in2t = nc.declare_dram_parameter("in2t", [NB, 2, 128, 3, 512], f32r, isOutput=False)
    in2n = nc.declare_dram_parameter("in2n", [NB, 128, NS, NA], bf16, isOutput=False)
    wts = nc.declare_dram_parameter("wts", [2, 128, 3, DP], f32r, isOutput=False)
    out_h = [
        nc.declare_dram_parameter("out_a", [NB, T // 512, 128, 4, D], f32, isOutput=True),
        nc.declare_dram_parameter("out_b", [NB, T // 512, 128, 4, D], f32, isOutput=True),
    ]

    with TileContext(nc) as tc:
        with (
            tc.tile_pool(name="wpool", bufs=1) as wpool,
            tc.tile_pool(name="a1p", bufs=2) as a1p,
            tc.tile_pool(name="a2p", bufs=2) as a2p,
            tc.tile_pool(name="a2np", bufs=2) as a2np,
            tc.tile_pool(name="projp", bufs=2) as projp,
            tc.tile_pool(name="ep", bufs=2) as ep,
            tc.tile_pool(name="outp", bufs=3) as outp,
            tc.tile_pool(name="recp", bufs=4) as recp,
            tc.tile_pool(name="ps_pj", bufs=2, space="PSUM") as ps_pj,
            tc.tile_pool(name="ps_sc", bufs=3, space="PSUM") as ps_sc,
            tc.tile_pool(name="ps_o", bufs=2, space="PSUM") as ps_o,
        ):
            # Weights: [p, attn, kd, e] — one DMA, resident all kernel.
            wt = wpool.tile([128, 2, 3, DP], f32r)
            nc.sync.dma_start(out=wt[:, 0, :, :], in_=wts[0])

            for lb in range(NB):
                # Load order matters for the pipeline head: mm1 needs only
                # a2 (+wt); mm2 then consumes a1 chunk-by-chunk; a2n is not
                # needed until the first mm4 (~15us of PE work later).
                a2 = a2p.tile([128, 2, 3, 512], f32r)
                for h in range(2):
                    nc.sync.dma_start(out=a2[:, h], in_=in2t[lb, h])
                if lb == 0:
                    nc.sync.dma_start(out=wt[:, 1, :, :], in_=wts[1])
                a1 = a1p.tile([128, 3, T], f32r)
                for c in range(3):
                    nc.sync.dma_start(
                        out=a1[:, c, :],
                        in_=in1t[lb, c * 128 : (c + 1) * 128, :],
                    )
                a2n = a2np.tile([128, NS, NA], bf16)
                nc.sync.dma_start(out=a2n, in_=in2n[lb])

                for a in range(2):
                    # mm1: projT [e, s] in 3 e-chunks of 128.
                    pt = projp.tile([128, 3, S], f32r)
                    for h in range(S // 512):
                        for ke in range(3):
                            pj = ps_pj.tile([128, 512], f32)
                            for kd in range(3):
                                nc.tensor.matmul(
                                    pj,
                                    wt[:, a, kd, ke * 128 : (ke + 1) * 128],
                                    a2[:, h, kd, :],
                                    start=(kd == 0),
                                    stop=(kd == 2),
                                )
                            nc.vector.tensor_copy(
                                pt[:, ke, h * 512 : (h + 1) * 512], pj
                            )

                    for tb in range(NT):
                        # mm2 + exp: E[s, t] for this 512-wide t-block.
                        E = ep.tile([128, NS, 512], bf16)
                        for st in range(NS):
                            sc = ps_sc.tile([128, 512], f32)
                            for ke in range(3):
                                nc.tensor.matmul(
                                    sc,
                                    pt[:, ke, st * 128 : (st + 1) * 128],
                                    a1[:, ke, tb * 512 : (tb + 1) * 512],
                                    start=(ke == 0),
                                    stop=(ke == 2),
                                )
                            nc.scalar.activation(out=E[:, st, :], in_=sc, func=Exp)

                        # mm4 + normalize: out[t, :300] and denominator col 300.
                        ostg = outp.tile([128, 4, D], f32)
                        for ts in range(4):
                            o = ps_o.tile([128, NA], f32)
                            for st in range(NS):
                                nc.tensor.matmul(
                                    o,
                                    E[:, st, ts * 128 : (ts + 1) * 128],
                                    a2n[:, st, :],
                                    start=(st == 0),
                                    stop=(st == NS - 1),
                                )
                            rec = recp.tile([128, 1], f32)
                            nc.vector.reciprocal(rec, o[:, 300:301])
                            nc.vector.tensor_scalar_mul(
                                ostg[:, ts, :], o[:, 0:D], rec
                            )
                        if lb == NB - 1 and a == 1 and tb == NT - 1:
                            for ts in range(4):
                                nc.sync.dma_start(
                                    out=out_h[a][lb, tb, :, ts], in_=ostg[:, ts]
                                )
                        else:
                            nc.sync.dma_start(out=out_h[a][lb, tb], in_=ostg)
    _split_multi_waits(nc)
    return nc


def kernel(input1, input2, W2, b2, W3, b3, mode=None, **_ignored):
    from concourse.bass_utils import run_bass_kernel_spmd

    input1 = np.asarray(input1, dtype=np.float32)
    input2 = np.asarray(input2, dtype=np.float32)
    W2 = np.asarray(W2, dtype=np.float32)
    W3 = np.asarray(W3, dtype=np.float32)
    # bias b2/b3 add a per-(b,t) constant to the softmax logits — no effect.

    if "nc" not in _CACHE:
        _CACHE["nc"] = _build_nc()
    nc = _CACHE["nc"]

    in1t = np.zeros((B, DP, T), np.float32)
    in1t[:, :D, :] = input1.transpose(0, 2, 1)
    in2t = np.zeros((B, DP, S), np.float32)
    in2t[:, :D, :] = input2.transpose(0, 2, 1)
    # [B, d(c*128+p), s(h*512+j)] -> [B, h, p, c, j]
    in2t = np.ascontiguousarray(
        in2t.reshape(B, 3, 128, 2, 512).transpose(0, 3, 2, 1, 4)
    )
    import ml_dtypes

    in2n = np.zeros((B, S, NA), np.float32)
    in2n[:, :, :D] = input2
    in2n[:, :, 300] = 1.0
    in2n = np.ascontiguousarray(
        in2n.reshape(B, S // 128, 128, NA).transpose(0, 2, 1, 3)
    ).astype(ml_dtypes.bfloat16)
    wts = np.zeros((2, DP, DP), np.float32)
    wts[0, :D, :D] = W2.T
    wts[1, :D, :D] = W3.T
    wts = np.ascontiguousarray(wts.reshape(2, 3, 128, DP).transpose(0, 2, 1, 3))

    in_maps = [
        {
            "in1t": np.ascontiguousarray(in1t[c * NB : (c + 1) * NB]),
            "in2t": np.ascontiguousarray(in2t[c * NB : (c + 1) * NB]),
            "in2n": np.ascontiguousarray(in2n[c * NB : (c + 1) * NB]),
            "wts": wts,
        }
        for c in range(NCORES)
    ]

    trace = bool(int(os.environ.get("KERNEL_PROFILE", "0")))
    if trace:
        _install_profile_hook()
    res = run_bass_kernel_spmd(nc, in_maps, list(range(NCORES)), trace=trace)
    _CACHE["last_exec_time_ns"] = res.exec_time_ns
    _CACHE["last_results"] = res

    def unswizzle(name):
        arr = np.concatenate([res.results[c][name] for c in range(NCORES)], axis=0)
        # [B, T//512, 128(p), 4(ts), D] -> [B, T, D] with t = tb*512 + ts*128 + p
        return np.ascontiguousarray(
            arr.transpose(0, 1, 3, 2, 4).reshape(B, T, D)
        )

    return unswizzle("out_a"), unswizzle("out_b")
